# revision 14
# baseline (speedup 1.0000x reference)
"""DMN forward on 8 Trainium2 NeuronCores (Bass/Tile), bf16 matmul build.

Sharding: batch rows 8/core for fact+question encoding and episodic memory
(core j owns batch rows 8j..8j+7 and their 160 fact sequences); decode GRU
replicated on all cores, fc/log-softmax vocab-sharded 4000 columns/core, with
one small AllGather per decode step carrying (max, argmax-token, expsum) for
the greedy feedback and the log-softmax normalizer.

All matmuls run in bf16 (PE single-pass) with fp32 PSUM accumulation; GRU
non-linearities use the native Sigmoid table in the encoder and the tanh form
in decode (so Exp shares the same table).  Each GRU step packs its six gate
blocks into two PSUM banks (r/z and n) as column groups, so the elementwise
tail is ~9 fused ops.  The fact-token projection is hoisted, SBUF-resident
(bf16), biases folded in at PSUM-evict time, and its gathers/transposes/
matmuls are emitted software-pipelined with the fact GRU steps.  Decode is
staggered: the recurrent-path matmuls of step t+1, the exp/normalizer work
(lagged one step), and the previous step's output writes all execute inside
step t's AllGather window.  The log-softmax normalizer uses a bitcast
fast-log plus one exp-Newton step instead of Ln, keeping the scalar engine
on a single activation table through decode.
"""

import os
import numpy as np

import concourse.bass as bass
import concourse.bacc as bacc
import concourse.mybir as mybir
from concourse.tile import TileContext
from concourse.bass_utils import run_bass_kernel_spmd
from concourse.masks import make_identity

AF = mybir.ActivationFunctionType
ALU = mybir.AluOpType
DT = mybir.dt
BF = DT.bfloat16
F32 = DT.float32

V, E, H = 32000, 256, 256
B, NF, FL, QL = 64, 20, 32, 16
N_EPISODE = 3
SEQBEGIN = 1
NCORE = 8
BB = B // NCORE            # batch rows per core = 8
NSEQ = BB * NF             # fact seqs per core = 160
NTOK = NSEQ * FL           # fact tokens per core = 5120
VS = V // NCORE            # vocab shard = 4000
NCHUNK = 8
CHW = VS // NCHUNK         # 500
FH = NSEQ // 2             # fact chain width = 80
VH = VS // 2               # argmax half-scan width = 2000

GK = {"ig": E, "qg": E, "att": H, "mem": H, "ans": 2 * H}

# fast-log constants: ln(x) ~ (bitcast_i32(x) - LOGB) * LOGK, refined by one
# Newton step y <- y + (x*exp(-y) - 1)
LOGK = float(np.log(2.0) / (1 << 23))
LOGB = 1064866805.0


def build_nc(alen, fcb_nonzero):
    nc = bacc.Bacc("TRN2", num_devices=NCORE)

    def dram_in(name, shape, dtype=F32):
        return nc.dram_tensor(name, list(shape), dtype, kind="ExternalInput")

    io = {}
    io["facts_idx"] = dram_in("facts_idx", [128, NTOK // 128], DT.int32)
    io["q_idx"] = dram_in("q_idx", [BB * QL, 1], DT.int32)
    io["embed"] = dram_in("embed", [V, E], BF)
    io["fcwT"] = dram_in("fcwT", [E, VS], BF)
    io["l0"] = dram_in("l0", [128, 2 * B], BF)
    io["voff"] = dram_in("voff", [B, 1])
    if fcb_nonzero:
        io["fcb"] = dram_in("fcb", [B, VS])
    for g, kin in GK.items():
        io[f"{g}_wihT"] = dram_in(f"{g}_wihT", [kin, 3 * H], BF)
        io[f"{g}_whhT"] = dram_in(f"{g}_whhT", [H, 3 * H], BF)
        io[f"{g}_brz6"] = dram_in(f"{g}_brz6", [128, 6])
        io[f"{g}_bhn"] = dram_in(f"{g}_bhn", [128, 2])
    io["g1T"] = dram_in("g1T", [4 * H, H], BF)
    io["g2T"] = dram_in("g2T", [H, 1], BF)
    io["gb1"] = dram_in("gb1", [128, 2])
    io["gb2"] = dram_in("gb2", [1, 1])
    io["memrz"] = dram_in("memrz", [128, 4 * BB])
    io["membin"] = dram_in("membin", [128, 2 * BB])

    out_logp = nc.dram_tensor("out_logp", [B * alen, VS], F32, kind="ExternalOutput")

    cc_enc_in = nc.dram_tensor("cc_enc_in", [BB, 2 * H], F32, kind="Internal")
    cc_enc_out = nc.dram_tensor("cc_enc_out", [B, 2 * H], F32, kind="Internal", addr_space="Shared")
    n_cc = alen + 1
    cc_top_in = [nc.dram_tensor(f"cc_top_in{t}", [B, 4], F32, kind="Internal") for t in range(n_cc)]
    cc_top_out = [nc.dram_tensor(f"cc_top_out{t}", [NCORE * B, 4], F32, kind="Internal",
                                 addr_space="Shared") for t in range(n_cc)]
    rg = [list(range(NCORE))]

    dbg = int(os.environ.get("K_DEBUG_STEPS", "0"))
    n_fl = dbg or FL
    n_ql = dbg or QL
    n_nf = dbg or NF
    n_ep = 1 if dbg else N_EPISODE
    n_dec = min(alen, dbg) if dbg else alen

    with TileContext(nc) as tc:
        with tc.tile_pool(name="shared", bufs=1) as shp, \
             tc.tile_pool(name="state", bufs=1) as st, \
             tc.tile_pool(name="work", bufs=3) as wk:

            identB = shp.tile([128, 128], BF)
            make_identity(nc, identB[:, :])
            identF = shp.tile([128, 128], F32)
            make_identity(nc, identF[:, :])

            W = {}

            def load_gate(pool, g):
                kin = GK[g]
                xw = []
                for k in range(kin // 128):
                    t = pool.tile([128, 3 * H], BF, name=f"{g}xw{k}")
                    nc.sync.dma_start(t[:, :], io[f"{g}_wihT"][k * 128:(k + 1) * 128, :])
                    xw.append(t)
                hw = []
                for k in range(2):
                    t = pool.tile([128, 3 * H], BF, name=f"{g}hw{k}")
                    nc.sync.dma_start(t[:, :], io[f"{g}_whhT"][k * 128:(k + 1) * 128, :])
                    hw.append(t)
                brz = pool.tile([128, 6], F32, name=f"{g}brz")
                nc.sync.dma_start(brz[:, :], io[f"{g}_brz6"][:, :])
                bhn = pool.tile([128, 2], F32, name=f"{g}bhn")
                nc.sync.dma_start(bhn[:, :], io[f"{g}_bhn"][:, :])
                W[g] = (xw, hw, brz, bhn)

            evict_rr = [0]

            def evict(dst_ap, src_ap, bias=None):
                if bias is None:
                    if evict_rr[0] % 2 == 0:
                        nc.vector.tensor_copy(dst_ap, src_ap)
                    else:
                        nc.scalar.activation(dst_ap, src_ap, AF.Copy)
                else:
                    if evict_rr[0] % 2 == 0:
                        nc.vector.tensor_scalar(dst_ap, src_ap, bias, None, ALU.add)
                    else:
                        nc.scalar.activation(dst_ap, src_ap, AF.Identity, bias=bias)
                evict_rr[0] += 1

            # ---- one fused GRU step -------------------------------------
            # brz: psum [128, 4F] (r0 r1 z0 z1), bnh: psum [128, 2F] (n0 n1)
            # h: [128, 2F] bf16 (hidden half k at cols kF).
            def gru_mms(brz, bnh, g, h_ap, F, x_rhs=None, xw_override=None):
                xw, hw, _, _ = W[g]
                if xw_override is not None:
                    xw = xw_override

                def dst(m):
                    if m < 4:
                        return brz[:, m * F:(m + 1) * F]
                    return bnh[:, (m - 4) * F:(m - 3) * F]

                for m in range(6):
                    first = True
                    if x_rhs is not None:
                        for k in range(len(x_rhs)):
                            nc.tensor.matmul(dst(m), xw[k][:, m * 128:(m + 1) * 128],
                                             x_rhs[k], start=first, stop=False)
                            first = False
                    for k in range(2):
                        nc.tensor.matmul(dst(m), hw[k][:, m * 128:(m + 1) * 128],
                                         h_ap[:, k * F:(k + 1) * F],
                                         start=first, stop=(k == 1))
                        first = False

            def gru_ew(brz, bnh, g, h_ap, F, gi_rz, gi_n, name, sig_direct=True,
                       att=None):
                """gi_rz: AP [128, 4F]-size (x-proj + all r/z biases);
                gi_n: AP [128, 2F]-size (x-proj n + bih_n).
                att: None for plain GRU; (e_ap, negg_ap) for episode form."""
                _, _, _, bhn = W[g]
                trz = wk.tile([128, 4 * F], BF, tag=f"trz{F}{name[0]}", bufs=2,
                              name=f"{name}trz")
                nc.vector.tensor_add(trz[:, :].rearrange("p (m f) -> p m f", f=F),
                                     brz[:, 0:4 * F].rearrange("p (m f) -> p m f", f=F),
                                     gi_rz)
                rz = wk.tile([128, 4 * F], BF, tag=f"rz{F}{name[0]}", bufs=2,
                             name=f"{name}rz")
                if sig_direct:
                    nc.scalar.activation(rz[:, :], trz[:, :], AF.Sigmoid)
                else:
                    th = wk.tile([128, 4 * F], BF, tag=f"th{F}{name[0]}", bufs=2,
                                 name=f"{name}th")
                    nc.scalar.activation(th[:, :], trz[:, :], AF.Tanh, scale=0.5)
                    nc.vector.tensor_scalar(rz[:, :], th[:, :], 0.5, 0.5,
                                            ALU.mult, ALU.add)
                y = wk.tile([128, 2 * F], BF, tag=f"y{F}{name[0]}", bufs=2,
                            name=f"{name}y")
                for h in range(2):
                    nc.vector.scalar_tensor_tensor(
                        y[:, h * F:(h + 1) * F], bnh[:, h * F:(h + 1) * F],
                        bhn[:, h:h + 1], rz[:, h * F:(h + 1) * F], ALU.add, ALU.mult)
                u = wk.tile([128, 2 * F], BF, tag=f"u{F}{name[0]}", bufs=2,
                            name=f"{name}u")
                nc.vector.tensor_add(u[:, :].rearrange("p (m f) -> p m f", f=F),
                                     y[:, :].rearrange("p (m f) -> p m f", f=F), gi_n)
                n = wk.tile([128, 2 * F], BF, tag=f"n{F}{name[0]}", bufs=2,
                            name=f"{name}n")
                nc.scalar.activation(n[:, :], u[:, :], AF.Tanh)
                if att is None:
                    # h' = n + z * (h - n)
                    d = wk.tile([128, 2 * F], BF, tag=f"d{F}{name[0]}", bufs=2,
                                name=f"{name}d")
                    nc.vector.tensor_sub(d[:, :], h_ap, n[:, :])
                    w2 = wk.tile([128, 2 * F], BF, tag=f"w{F}{name[0]}", bufs=2,
                                 name=f"{name}w")
                    nc.vector.tensor_mul(w2[:, :], rz[:, 2 * F:4 * F], d[:, :])
                    hn = wk.tile([128, 2 * F], BF, tag=f"hn{F}{name[0]}", bufs=2,
                                 name=f"{name}hn")
                    nc.vector.tensor_add(hn[:, :], n[:, :], w2[:, :])
                    return hn
                else:
                    # e' = e + g*(1-z)*(n - e);  w1 = (z - 1)*(-g)
                    e_ap, negg = att
                    d = wk.tile([128, 2 * F], BF, tag=f"d{F}{name[0]}", bufs=2,
                                name=f"{name}d")
                    nc.vector.tensor_sub(d[:, :], n[:, :], e_ap)
                    w1 = wk.tile([128, 2 * F], BF, tag=f"w1{F}{name[0]}", bufs=2,
                                 name=f"{name}w1")
                    nc.vector.scalar_tensor_tensor(
                        w1[:, :].rearrange("p (m f) -> p m f", f=F),
                        rz[:, 2 * F:4 * F].rearrange("p (m f) -> p m f", f=F),
                        1.0, negg, ALU.subtract, ALU.mult)
                    p2 = wk.tile([128, 2 * F], BF, tag=f"p2{F}{name[0]}", bufs=2,
                                 name=f"{name}p2")
                    nc.vector.tensor_mul(p2[:, :], w1[:, :], d[:, :])
                    en = wk.tile([128, 2 * F], BF, tag=f"hn{F}{name[0]}", bufs=2,
                                 name=f"{name}en")
                    nc.vector.tensor_add(en[:, :], e_ap, p2[:, :])
                    return en

            # ========== P1+P2+P3: pipelined gather/x-proj + GRUs =========
            with tc.tile_pool(name="fpool", bufs=1) as fp, \
                 tc.tile_pool(name="psf", bufs=1, space="PSUM") as psf:
                load_gate(fp, "ig")
                load_gate(fp, "qg")
                XT = [fp.tile([128, NTOK], BF, name=f"XT{k}") for k in range(2)]
                fidx = fp.tile([128, NTOK // 128], DT.int32, name="fidx")
                nc.sync.dma_start(fidx[:, :], io["facts_idx"][:, :])
                qidx = wk.tile([128, 1], DT.int32, name="qidx")
                nc.sync.dma_start(qidx[:, :], io["q_idx"][:, :])

                gi = fp.tile([128, 6 * NTOK], BF, name="gi")
                gi3 = gi[:, :].rearrange("p (m t) -> p m t", t=NTOK)
                ig_brz = W["ig"][2]

                # question gather + x-projection first (tiny, unblocks q GRU)
                qg_t = wk.tile([128, E], BF, tag="fgat", bufs=4, name="qgat")
                nc.gpsimd.indirect_dma_start(
                    out=qg_t[:, :], out_offset=None, in_=io["embed"][:, :],
                    in_offset=bass.IndirectOffsetOnAxis(ap=qidx[:, :1], axis=0),
                )
                XQ = fp.tile([128, 2 * BB * QL], BF, name="XQ")
                for ch in range(2):
                    pt = psf.tile([128, 128], BF, tag="xpt", bufs=3, name=f"qtp{ch}")
                    nc.tensor.transpose(pt[:, :], qg_t[:, ch * 128:(ch + 1) * 128], identB[:, :])
                    evict(XQ[:, ch * 128:(ch + 1) * 128], pt[:, :])
                giq = fp.tile([128, 6 * BB * QL], BF, name="giq")
                giq3 = giq[:, :].rearrange("p (m t) -> p m t", t=BB * QL)
                qg_brz = W["qg"][2]
                for m in range(6):
                    pp = psf.tile([128, BB * QL], F32, tag="xpt", bufs=3, name=f"qxp{m}")
                    for k in range(2):
                        nc.tensor.matmul(pp[:, :], W["qg"][0][k][:, m * 128:(m + 1) * 128],
                                         XQ[:, k * 128:(k + 1) * 128], start=(k == 0), stop=(k == 1))
                    evict(giq3[:, m, :], pp[:, :], bias=qg_brz[:, m:m + 1])

                NG = NTOK // 128     # 40 gathers
                NCH = NTOK // 512    # 10 x-proj chunks

                def emit_gather(i):
                    gt = wk.tile([128, E], BF, tag="fgat", bufs=4, name=f"fg{i}")
                    nc.gpsimd.indirect_dma_start(
                        out=gt[:, :], out_offset=None, in_=io["embed"][:, :],
                        in_offset=bass.IndirectOffsetOnAxis(ap=fidx[:, i:i + 1], axis=0),
                    )
                    for ch in range(2):
                        pt = psf.tile([128, 128], BF, tag="xpt", bufs=3, name=f"ftp{i}_{ch}")
                        nc.tensor.transpose(pt[:, :], gt[:, ch * 128:(ch + 1) * 128], identB[:, :])
                        evict(XT[ch][:, i * 128:(i + 1) * 128], pt[:, :])

                def emit_xchunk(c):
                    for m in range(6):
                        pp = psf.tile([128, 512], F32, tag="xpt", bufs=3, name=f"xp{m}_{c}")
                        for k in range(2):
                            nc.tensor.matmul(pp[:, :], W["ig"][0][k][:, m * 128:(m + 1) * 128],
                                             XT[k][:, c * 512:(c + 1) * 512],
                                             start=(k == 0), stop=(k == 1))
                        evict(gi3[:, m, c * 512:(c + 1) * 512], pp[:, :],
                              bias=ig_brz[:, m:m + 1])

                g_done = 0
                c_done = 0

                hA = wk.tile([128, NSEQ], BF, tag="hA", bufs=2, name="hA0")
                hB = wk.tile([128, NSEQ], BF, tag="hB", bufs=2, name="hB0")
                hq = wk.tile([128, 2 * BB], BF, tag="hq", bufs=2, name="hq0")
                nc.vector.memset(hA[:, :], 0.0)
                nc.vector.memset(hB[:, :], 0.0)
                nc.vector.memset(hq[:, :], 0.0)
                for t in range(n_fl):
                    # stay ~3 GRU steps ahead with gathers / x-proj chunks
                    need_tok = min(NTOK, (t + 3) * NSEQ)
                    while g_done < NG and g_done * 128 < min(NTOK, need_tok + 512):
                        emit_gather(g_done)
                        g_done += 1
                    while c_done < NCH and c_done * 512 < need_tok:
                        emit_xchunk(c_done)
                        c_done += 1
                    bA = psf.tile([128, 6 * FH], F32, tag="Ab", bufs=2, name=f"bA{t}")
                    bArz, bAnh = bA[:, 0:4 * FH], bA[:, 4 * FH:6 * FH]
                    gru_mms(bArz, bAnh, "ig", hA[:, :], FH)
                    bB = psf.tile([128, 6 * FH], F32, tag="Bb", bufs=2, name=f"bB{t}")
                    bBrz, bBnh = bB[:, 0:4 * FH], bB[:, 4 * FH:6 * FH]
                    gru_mms(bBrz, bBnh, "ig", hB[:, :], FH)
                    do_q = (t % 2 == 0) and (t // 2 < n_ql)
                    if do_q:
                        tq = t // 2
                        bQ = psf.tile([128, 6 * BB], F32, tag="Qb", bufs=1, name=f"bQ{tq}")
                        bQrz, bQnh = bQ[:, 0:4 * BB], bQ[:, 4 * BB:6 * BB]
                        gru_mms(bQrz, bQnh, "qg", hq[:, :], BB)
                    t0 = t * NSEQ
                    hA = gru_ew(bArz, bAnh, "ig", hA[:, :], FH,
                                gi3[:, 0:4, t0:t0 + FH], gi3[:, 4:6, t0:t0 + FH],
                                name=f"A{t}_")
                    hB = gru_ew(bBrz, bBnh, "ig", hB[:, :], FH,
                                gi3[:, 0:4, t0 + FH:t0 + NSEQ],
                                gi3[:, 4:6, t0 + FH:t0 + NSEQ], name=f"B{t}_")
                    if do_q:
                        q0 = tq * BB
                        hq = gru_ew(bQrz, bQnh, "qg", hq[:, :], BB,
                                    giq3[:, 0:4, q0:q0 + BB], giq3[:, 4:6, q0:q0 + BB],
                                    name=f"Q{tq}_")

                # persist enc_facts [128, 2, NSEQ] and enc_q [128, 2*BB]
                encf = st.tile([128, 2 * NSEQ], BF, name="encf")
                for h in range(2):
                    nc.vector.tensor_copy(encf[:, h * NSEQ:h * NSEQ + FH],
                                          hA[:, h * FH:(h + 1) * FH])
                    nc.vector.tensor_copy(encf[:, h * NSEQ + FH:(h + 1) * NSEQ],
                                          hB[:, h * FH:(h + 1) * FH])
                hqF = st.tile([128, 2 * BB], BF, name="hqF")
                nc.vector.tensor_copy(hqF[:, :], hq[:, :])

            # ================= P4: episodes =================
            with tc.tile_pool(name="epool", bufs=1) as epl, \
                 tc.tile_pool(name="pse", bufs=1, space="PSUM") as pse:
                load_gate(epl, "att")
                load_gate(epl, "mem")
                g1T = []
                for k in range(8):
                    tt = epl.tile([128, H], BF, name=f"g1T{k}")
                    nc.sync.dma_start(tt[:, :], io["g1T"][k * 128:(k + 1) * 128, :])
                    g1T.append(tt)
                g2T = []
                for k in range(2):
                    tt = epl.tile([128, 1], BF, name=f"g2T{k}")
                    nc.sync.dma_start(tt[:, :], io["g2T"][k * 128:(k + 1) * 128, :])
                    g2T.append(tt)
                gb1 = epl.tile([128, 2], F32)
                nc.sync.dma_start(gb1[:, :], io["gb1"][:, :])
                gb2 = epl.tile([1, 1], F32)
                nc.sync.dma_start(gb2[:, :], io["gb2"][:, :])
                memrz = epl.tile([128, 4 * BB], F32)
                nc.sync.dma_start(memrz[:, :], io["memrz"][:, :])
                membin = epl.tile([128, 2 * BB], F32)
                nc.sync.dma_start(membin[:, :], io["membin"][:, :])

                # att x-projection of enc_facts (+ biases)
                giaP = epl.tile([128, 6 * NSEQ], BF, name="giaP")
                gia3 = giaP[:, :].rearrange("p (m t) -> p m t", t=NSEQ)
                att_brz = W["att"][2]
                for m in range(6):
                    pp = pse.tile([128, NSEQ], F32, tag="xp2", bufs=2, name=f"axp{m}")
                    for k in range(2):
                        nc.tensor.matmul(pp[:, :], W["att"][0][k][:, m * 128:(m + 1) * 128],
                                         encf[:, k * NSEQ:(k + 1) * NSEQ],
                                         start=(k == 0), stop=(k == 1))
                    evict(gia3[:, m, :], pp[:, :], bias=att_brz[:, m:m + 1])
                gia4 = giaP[:, :].rearrange("p (m b i) -> p m b i", m=6, i=NF)

                memT = wk.tile([128, 2 * BB], BF, tag="memT", bufs=2, name="memT0")
                nc.vector.tensor_copy(memT[:, :], hqF[:, :])
                encf3 = [encf[:, k * NSEQ:(k + 1) * NSEQ].rearrange("p (r i) -> p r i", i=NF)
                         for k in range(2)]

                for ep in range(n_ep):
                    # gate features z = [f*q, f*m, |f-q|, |f-m|]
                    ZT = [wk.tile([128, NSEQ], BF, tag=f"zt{x}", bufs=1, name=f"ZT{ep}_{x}")
                          for x in range(8)]
                    for h in range(2):
                        qb = hqF[:, h * BB:(h + 1) * BB].to_broadcast([128, BB, NF])
                        mb = memT[:, h * BB:(h + 1) * BB].to_broadcast([128, BB, NF])
                        z3 = [ZT[x][:, :].rearrange("p (r i) -> p r i", i=NF) for x in range(8)]
                        nc.vector.tensor_mul(z3[0 + h], encf3[h], qb)
                        nc.vector.tensor_mul(z3[2 + h], encf3[h], mb)
                        dq = wk.tile([128, NSEQ], F32, tag="dq", bufs=2, name=f"dq{ep}_{h}")
                        nc.vector.tensor_sub(dq[:, :].rearrange("p (r i) -> p r i", i=NF),
                                             encf3[h], qb)
                        nc.scalar.activation(ZT[4 + h][:, :], dq[:, :], AF.Abs)
                        dm = wk.tile([128, NSEQ], F32, tag="dm", bufs=2, name=f"dm{ep}_{h}")
                        nc.vector.tensor_sub(dm[:, :].rearrange("p (r i) -> p r i", i=NF),
                                             encf3[h], mb)
                        nc.scalar.activation(ZT[6 + h][:, :], dm[:, :], AF.Abs)
                    p1T = []
                    for m in range(2):
                        pp = pse.tile([128, NSEQ], F32, tag="p1", bufs=2, name=f"p1{ep}_{m}")
                        for k in range(8):
                            nc.tensor.matmul(pp[:, :], g1T[k][:, m * 128:(m + 1) * 128],
                                             ZT[k][:, :], start=(k == 0), stop=(k == 7))
                        t1 = wk.tile([128, NSEQ], BF, tag="p1s", bufs=2, name=f"p1s{ep}_{m}")
                        nc.scalar.activation(t1[:, :], pp[:, :], AF.Tanh, bias=gb1[:, m:m + 1])
                        p1T.append(t1)
                    pgp = pse.tile([1, NSEQ], F32, tag="pg", bufs=1, name=f"pg{ep}")
                    for k in range(2):
                        nc.tensor.matmul(pgp[:, :], g2T[k][:, :], p1T[k][:, :],
                                         start=(k == 0), stop=(k == 1))
                    g_row = wk.tile([1, NSEQ], F32, tag="grow", bufs=1, name=f"grow{ep}")
                    nc.scalar.activation(g_row[:, :], pgp[:, :], AF.Sigmoid, bias=gb2[:1, :1])
                    ngrow = wk.tile([1, NSEQ], F32, tag="ngrow", bufs=1, name=f"ngrow{ep}")
                    nc.vector.tensor_scalar_mul(ngrow[:, :], g_row[:, :], -1.0)
                    negG = wk.tile([128, 2 * NSEQ], F32, tag="negG", bufs=1, name=f"negG{ep}")
                    nc.gpsimd.partition_broadcast(negG[:, 0:NSEQ], ngrow[:, :])
                    nc.vector.tensor_copy(negG[:, NSEQ:2 * NSEQ], negG[:, 0:NSEQ])
                    negG4 = negG[:, :].rearrange("p (h b i) -> p h b i", h=2, i=NF)

                    eT = wk.tile([128, 2 * BB], BF, tag="eT", bufs=2, name=f"eT{ep}")
                    nc.vector.memset(eT[:, :], 0.0)
                    for i in range(n_nf):
                        eb = pse.tile([128, 6 * BB], F32, tag="eb", bufs=2, name=f"eb{ep}_{i}")
                        erz, enh = eb[:, 0:4 * BB], eb[:, 4 * BB:6 * BB]
                        gru_mms(erz, enh, "att", eT[:, :], BB)
                        eT = gru_ew(erz, enh, "att", eT[:, :], BB,
                                    gia4[:, 0:4, :, i], gia4[:, 4:6, :, i],
                                    name=f"e{ep}_{i}_", att=(eT[:, :], negG4[:, :, :, i]))
                    # memory GRU step (x = eT)
                    mb = pse.tile([128, 6 * BB], F32, tag="eb", bufs=2, name=f"mb{ep}")
                    mrz, mnh = mb[:, 0:4 * BB], mb[:, 4 * BB:6 * BB]
                    gru_mms(mrz, mnh, "mem", memT[:, :], BB,
                            x_rhs=[eT[:, k * BB:(k + 1) * BB] for k in range(2)])
                    memT = gru_ew(mrz, mnh, "mem", memT[:, :], BB,
                                  memrz[:, :].rearrange("p (m f) -> p m f", f=BB),
                                  membin[:, :].rearrange("p (m f) -> p m f", f=BB),
                                  name=f"m{ep}_")

                memF = st.tile([128, 2 * BB], BF, name="memF")
                nc.vector.tensor_copy(memF[:, :], memT[:, :])

            # ================= P5+P6: all-gather mem|enc_q, decode =======
            with tc.tile_pool(name="dpool", bufs=1) as dp, \
                 tc.tile_pool(name="psd", bufs=1, space="PSUM") as psd:
                load_gate(dp, "ans")
                fcwT = []
                for k in range(2):
                    tt = dp.tile([128, VS], BF, name=f"fcwT{k}")
                    nc.sync.dma_start(tt[:, :], io["fcwT"][k * 128:(k + 1) * 128, :])
                    fcwT.append(tt)
                l0 = dp.tile([128, 2 * B], BF, name="l0")
                nc.sync.dma_start(l0[:, :], io["l0"][:, :])
                vofft = dp.tile([B, 1], F32)
                nc.sync.dma_start(vofft[:, :], io["voff"][:, :])
                iota_i = dp.tile([B, VS], DT.int32)
                nc.gpsimd.iota(iota_i[:, :], pattern=[[1, VS]], base=0, channel_multiplier=0)
                iotaG = dp.tile([B, VS], F32)
                nc.vector.tensor_copy(iotaG[:, :], iota_i[:, :])
                encrow = wk.tile([BB, 2 * H], F32, name="encrow")
                for h in range(2):
                    pt = psd.tile([BB, 128], BF, tag="misc", bufs=2, name=f"egm{h}")
                    nc.tensor.transpose(pt[:, :], memF[:, h * BB:(h + 1) * BB], identB[:, :])
                    evict(encrow[:, h * 128:(h + 1) * 128], pt[:, :])
                    pt2 = psd.tile([BB, 128], BF, tag="misc", bufs=2, name=f"egq{h}")
                    nc.tensor.transpose(pt2[:, :], hqF[:, h * BB:(h + 1) * BB], identB[:, :])
                    evict(encrow[:, 256 + h * 128:256 + (h + 1) * 128], pt2[:, :])
                nc.sync.dma_start(cc_enc_in[:, :], encrow[:, :])
                nc.gpsimd.collective_compute("AllGather", ALU.bypass, ins=[cc_enc_in[:, :]],
                                             outs=[cc_enc_out[:, :]], replica_groups=rg)
                enc_all = wk.tile([B, 2 * H], F32, name="enc_all")
                nc.sync.dma_start(enc_all[:, :], cc_enc_out[:, :])
                # transposed mem/enc_q for all 64 rows
                hid = wk.tile([128, 2 * B], BF, tag="hid", bufs=2, name="hid0")
                qA = dp.tile([128, 2 * B], BF, name="qA")
                for h in range(2):
                    pt = psd.tile([128, B], F32, tag="misc", bufs=2, name=f"tmA{h}")
                    nc.tensor.transpose(pt[:, :], enc_all[:, h * 128:(h + 1) * 128],
                                        identF[:B, :B])
                    evict(hid[:, h * B:(h + 1) * B], pt[:, :])
                    pt2 = psd.tile([128, B], F32, tag="misc", bufs=2, name=f"tqA{h}")
                    nc.tensor.transpose(pt2[:, :], enc_all[:, 256 + h * 128:256 + (h + 1) * 128],
                                        identF[:B, :B])
                    evict(qA[:, h * B:(h + 1) * B], pt2[:, :])

                # enc_q half of the ans input projection (+ all r/z/n biases)
                giq6 = dp.tile([128, 6 * B], BF, name="giq6")
                giq63 = giq6[:, :].rearrange("p (m t) -> p m t", t=B)
                ans_xw = W["ans"][0]
                ans_brz = W["ans"][2]
                for m in range(6):
                    pp = psd.tile([128, B], F32, tag="misc", bufs=2, name=f"dxp{m}")
                    for k in range(2):
                        nc.tensor.matmul(pp[:, :], ans_xw[2 + k][:, m * 128:(m + 1) * 128],
                                         qA[:, k * B:(k + 1) * B], start=(k == 0), stop=(k == 1))
                    evict(giq63[:, m, :], pp[:, :], bias=ans_brz[:, m:m + 1])

                lastT = l0
                out3 = out_logp.rearrange("(b t) v -> b t v", t=alen)
                stores = {}
                sxs_prev = None
                pending_write = None

                def write_step(wts, nlzneg):
                    sv = stores.pop(wts)
                    for vv in range(2):
                        sl = sv[:, vv * VH:(vv + 1) * VH]
                        ot = dp.tile([B, VH], F32, tag=f"ot{vv}", bufs=2,
                                     name=f"ot{wts}_{vv}")
                        if vv == 0:
                            nc.vector.tensor_scalar(ot[:, :], sl, nlzneg, None, ALU.add)
                        else:
                            nc.scalar.activation(ot[:, :], sl, AF.Identity, bias=nlzneg)
                        nc.sync.dma_start(out3[:, wts, vv * VH:(vv + 1) * VH], ot[:, :])

                def read_cc_and_resolve(rts, need_token):
                    """Read topall(rts); resolve winner token (if needed) and,
                    for rts>=1, the lagged normalizer of step rts-1 + write."""
                    topall = wk.tile([B, 4 * NCORE], F32, tag="topall", bufs=2,
                                     name=f"topall{rts}")
                    nc.sync.dma_start(
                        topall[:, :].rearrange("b (c v) -> b c v", v=4),
                        cc_top_out[rts].rearrange("(c b) v -> b c v", b=B),
                    )
                    t3 = topall[:, :].rearrange("b (c v) -> b c v", v=4)
                    newl = None
                    if need_token:
                        gv = wk.tile([B, 1], F32, tag="gv", bufs=2, name=f"gv{rts}")
                        nc.vector.tensor_reduce(gv[:, :], t3[:, :, 0],
                                                axis=mybir.AxisListType.X, op=ALU.max)
                        wtokf = wk.tile([B, NCORE], F32, tag="wtokf", bufs=2,
                                        name=f"wtokf{rts}")
                        nc.vector.scalar_tensor_tensor(wtokf[:, :], t3[:, :, 0], gv[:, :],
                                                       t3[:, :, 1], ALU.is_equal, ALU.mult)
                        wtok = wk.tile([B, 1], F32, tag="wtok", bufs=2, name=f"wtok{rts}")
                        nc.vector.tensor_reduce(wtok[:, :], wtokf[:, :],
                                                axis=mybir.AxisListType.X, op=ALU.max)
                        wtoki = wk.tile([B, 1], DT.int32, tag="wtoki", bufs=2,
                                        name=f"wtoki{rts}")
                        nc.vector.tensor_copy(wtoki[:, :], wtok[:, :])
                        lemb = wk.tile([B, E], BF, tag="lemb", bufs=2, name=f"lemb{rts}")
                        nc.gpsimd.indirect_dma_start(
                            out=lemb[:, :], out_offset=None, in_=io["embed"][:, :],
                            in_offset=bass.IndirectOffsetOnAxis(ap=wtoki[:, :1], axis=0),
                        )
                        newl = wk.tile([128, 2 * B], BF, tag="lastT", bufs=2,
                                       name=f"lastT{rts}")
                        for h in range(2):
                            pt = psd.tile([128, B], BF, tag="misc", bufs=2, name=f"lt{rts}_{h}")
                            nc.tensor.transpose(pt[:, :], lemb[:, h * 128:(h + 1) * 128],
                                                identB[:B, :B])
                            evict(newl[:, h * B:(h + 1) * B], pt[:, :])
                    pend = None
                    if rts >= 1:
                        # lagged normalizer for step rts-1
                        sxt = wk.tile([B, 1], F32, tag="sxt", bufs=2, name=f"sxt{rts}")
                        nc.vector.tensor_reduce(sxt[:, :], t3[:, :, 2],
                                                axis=mybir.AxisListType.X, op=ALU.add)
                        si = wk.tile([B, 1], F32, tag="si", bufs=2, name=f"si{rts}")
                        nc.vector.tensor_copy(si[:, :], sxt[:, :].bitcast(DT.int32))
                        y0 = wk.tile([B, 1], F32, tag="y0", bufs=2, name=f"y0{rts}")
                        nc.vector.tensor_scalar(y0[:, :], si[:, :], LOGB, LOGK,
                                                ALU.subtract, ALU.mult)
                        ee = wk.tile([B, 1], F32, tag="ee", bufs=2, name=f"ee{rts}")
                        nc.scalar.activation(ee[:, :], y0[:, :], AF.Exp, scale=-1.0)
                        zz = wk.tile([B, 1], F32, tag="zz", bufs=2, name=f"zz{rts}")
                        nc.vector.tensor_mul(zz[:, :], sxt[:, :], ee[:, :])
                        nlzneg = wk.tile([B, 1], F32, tag="nlzneg", bufs=2, name=f"nlz{rts}")
                        nc.vector.scalar_tensor_tensor(nlzneg[:, :], zz[:, :], 1.0, y0[:, :],
                                                       ALU.subtract, ALU.add)
                        nc.vector.tensor_scalar_mul(nlzneg[:, :], nlzneg[:, :], -1.0)
                        pend = (rts - 1, nlzneg[:, :])
                    return newl, pend

                xw_ans = ans_xw[:2]
                hw_ans = W["ans"][1]
                def emit_hmms(db_t, h_ap, sts):
                    for m in range(6):
                        dm_ = db_t[:, m * B:(m + 1) * B] if m < 4 else \
                            db_t[:, m * B:(m + 1) * B]
                        for k in range(2):
                            nc.tensor.matmul(db_t[:, m * B:(m + 1) * B],
                                             hw_ans[k][:, m * 128:(m + 1) * 128],
                                             h_ap[:, k * B:(k + 1) * B],
                                             start=(k == 0), stop=False,
                                             skip_group_check=True)

                db_next = None
                for ts in range(n_dec):
                    if ts == 0:
                        db = psd.tile([128, 6 * B], F32, tag="db", bufs=2, name=f"db{ts}")
                        emit_hmms(db, hid[:, :], ts)
                    else:
                        db = db_next
                        lastT, pending_write = read_cc_and_resolve(ts - 1, need_token=True)
                    drz, dnh = db[:, 0:4 * B], db[:, 4 * B:6 * B]

                    def ddst(m):
                        if m < 4:
                            return drz[:, m * B:(m + 1) * B]
                        return dnh[:, (m - 4) * B:(m - 3) * B]

                    for m in range(6):
                        for k in range(2):
                            nc.tensor.matmul(ddst(m), xw_ans[k][:, m * 128:(m + 1) * 128],
                                             lastT[:, k * B:(k + 1) * B],
                                             start=False, stop=(k == 1),
                                             skip_group_check=True)
                    hid = gru_ew(drz, dnh, "ans", hid[:, :], B,
                                 giq63[:, 0:4, :], giq63[:, 4:6, :],
                                 name=f"a{ts}_", sig_direct=False)

                    # --- fc + scan (copies on Act, chunk maxes on DVE);
                    # the argmax-index half-scans interleave with the chunks ---
                    store = dp.tile([B, VS], F32, tag="lst", bufs=3, name=f"lst{ts}")
                    stores[ts] = store[:, :]
                    Mt = wk.tile([B, NCHUNK], F32, tag="Mt", bufs=2, name=f"Mt{ts}")
                    Mh = wk.tile([B, 2], F32, tag="Mh", bufs=2, name=f"Mh{ts}")
                    It = wk.tile([B, 2], F32, tag="It", bufs=2, name=f"It{ts}")

                    def emit_half_scan(v):
                        nc.vector.tensor_reduce(Mh[:, v:v + 1], Mt[:, v * 4:(v + 1) * 4],
                                                axis=mybir.AxisListType.X, op=ALU.max)
                        mskh = dp.tile([B, VH], F32, tag="mskh", bufs=2,
                                       name=f"mskh{ts}_{v}")
                        nc.vector.scalar_tensor_tensor(
                            mskh[:, :], store[:, v * VH:(v + 1) * VH], Mh[:, v:v + 1],
                            iotaG[:, v * VH:(v + 1) * VH], ALU.is_equal, ALU.mult,
                            accum_out=It[:, v:v + 1])

                    for c in range(NCHUNK):
                        pl = psd.tile([B, CHW], F32, tag="fc", bufs=4, name=f"pl{ts}_{c}")
                        for k in range(2):
                            nc.tensor.matmul(pl[:, :], hid[:, k * B:(k + 1) * B],
                                             fcwT[k][:, c * CHW:(c + 1) * CHW],
                                             start=(k == 0), stop=(k == 1))
                        if fcb_nonzero:
                            fcbt = wk.tile([B, CHW], F32, tag="fcbt", bufs=2,
                                           name=f"fcbt{ts}_{c}")
                            nc.sync.dma_start(fcbt[:, :], io["fcb"][:, c * CHW:(c + 1) * CHW])
                            nc.vector.tensor_add(pl[:, :], pl[:, :], fcbt[:, :])
                        nc.scalar.activation(store[:, c * CHW:(c + 1) * CHW],
                                             pl[:, :], AF.Copy)
                        nc.vector.tensor_reduce(Mt[:, c:c + 1], pl[:, :],
                                                axis=mybir.AxisListType.X, op=ALU.max)
                        if c == 3:
                            emit_half_scan(0)
                    emit_half_scan(1)
                    gmax = wk.tile([B, 1], F32, tag="gmax", bufs=2, name=f"gmax{ts}")
                    nc.vector.tensor_reduce(gmax[:, :], Mh[:, :], axis=mybir.AxisListType.X,
                                            op=ALU.max)
                    wsel = wk.tile([B, 2], F32, tag="wsel", bufs=2, name=f"wsel{ts}")
                    nc.vector.scalar_tensor_tensor(wsel[:, :], Mh[:, :], gmax[:, :],
                                                   It[:, :], ALU.is_equal, ALU.mult)
                    tokf = wk.tile([B, 1], F32, tag="tokf", bufs=2, name=f"tokf{ts}")
                    nc.vector.tensor_reduce(tokf[:, :], wsel[:, :], axis=mybir.AxisListType.X,
                                            op=ALU.max)
                    pack = wk.tile([B, 4], F32, tag="pack", bufs=2, name=f"pack{ts}")
                    nc.vector.tensor_copy(pack[:, 0:1], gmax[:, :])
                    nc.vector.tensor_add(pack[:, 1:2], tokf[:, :], vofft[:, :])
                    if sxs_prev is not None:
                        nc.vector.tensor_copy(pack[:, 2:3], sxs_prev)
                    else:
                        nc.vector.memset(pack[:, 2:3], 0.0)
                    nc.vector.memset(pack[:, 3:4], 0.0)
                    nc.sync.dma_start(cc_top_in[ts][:, :], pack[:, :])

                    # pre-collective emission: everything below executes inside
                    # this step's CC window (the framework barriers on the
                    # collective, so post-collective work cannot overlap it)
                    if ts + 1 < n_dec:
                        db_next = psd.tile([128, 6 * B], F32, tag="db", bufs=2,
                                           name=f"db{ts + 1}")
                        emit_hmms(db_next, hid[:, :], ts + 1)
                    sx = wk.tile([B, NCHUNK], F32, tag="sx", bufs=2, name=f"sx{ts}")
                    for c in range(NCHUNK):
                        dump = dp.tile([B, CHW], BF, tag="dump", bufs=4, name=f"dump{ts}_{c}")
                        nc.scalar.activation(dump[:, :], store[:, c * CHW:(c + 1) * CHW],
                                             AF.Exp, accum_out=sx[:, c:c + 1])
                    sxs = wk.tile([B, 1], F32, tag="sxs", bufs=2, name=f"sxs{ts}")
                    nc.vector.tensor_reduce(sxs[:, :], sx[:, :], axis=mybir.AxisListType.X,
                                            op=ALU.add)
                    sxs_prev = sxs[:, :]
                    if pending_write is not None:
                        write_step(*pending_write)
                        pending_write = None
                    nc.gpsimd.collective_compute("AllGather", ALU.bypass,
                                                 ins=[cc_top_in[ts][:, :]],
                                                 outs=[cc_top_out[ts][:, :]], replica_groups=rg)

                # final collective: ship the last step's expsum
                packF = wk.tile([B, 4], F32, tag="pack", bufs=2, name="packF")
                nc.vector.memset(packF[:, 0:2], 0.0)
                nc.vector.tensor_copy(packF[:, 2:3], sxs_prev)
                nc.vector.memset(packF[:, 3:4], 0.0)
                nc.sync.dma_start(cc_top_in[n_dec][:, :], packF[:, :])
                nc.gpsimd.collective_compute("AllGather", ALU.bypass,
                                             ins=[cc_top_in[n_dec][:, :]],
                                             outs=[cc_top_out[n_dec][:, :]], replica_groups=rg)
                _, pw = read_cc_and_resolve(n_dec - 1, need_token=False)
                if pw is not None:
                    write_step(*pw)
                _, pw = read_cc_and_resolve(n_dec, need_token=False)
                if pw is not None:
                    write_step(*pw)

    nc.finalize()
    return nc


def prep_inputs(inputs):
    """Host-side shard/pack. Returns in_maps list for the 8 cores."""
    f32 = np.float32
    bfnp = DT.np(BF)
    emb = np.asarray(inputs["embed_w"], dtype=f32).astype(bfnp)
    packs = {}
    for g in GK:
        wih = np.asarray(inputs[f"{g}_wih"], dtype=f32)
        whh = np.asarray(inputs[f"{g}_whh"], dtype=f32)
        bih = np.asarray(inputs[f"{g}_bih"], dtype=f32)
        bhh = np.asarray(inputs[f"{g}_bhh"], dtype=f32)
        packs[f"{g}_wihT"] = np.ascontiguousarray(wih.T).astype(bfnp)
        packs[f"{g}_whhT"] = np.ascontiguousarray(whh.T).astype(bfnp)
        brz6 = np.empty((128, 6), f32)
        for m in range(4):
            brz6[:, m] = bih[m * 128:(m + 1) * 128] + bhh[m * 128:(m + 1) * 128]
        for hh in range(2):
            brz6[:, 4 + hh] = bih[512 + hh * 128:512 + (hh + 1) * 128]
        packs[f"{g}_brz6"] = brz6
        packs[f"{g}_bhn"] = np.ascontiguousarray(bhh[512:768].reshape(2, 128).T)
    packs["g1T"] = np.ascontiguousarray(np.asarray(inputs["gate_w1"], f32).T).astype(bfnp)
    packs["g2T"] = np.ascontiguousarray(np.asarray(inputs["gate_w2"], f32).T).astype(bfnp)
    packs["gb1"] = np.ascontiguousarray(np.asarray(inputs["gate_b1"], f32).reshape(2, 128).T)
    packs["gb2"] = np.asarray(inputs["gate_b2"], f32).reshape(1, 1)
    # memory-GRU constant bias tiles (x side has no hoisted projection)
    memb = packs["mem_brz6"]
    packs["memrz"] = np.ascontiguousarray(np.repeat(memb[:, 0:4], BB, axis=1), f32)
    packs["membin"] = np.ascontiguousarray(np.repeat(memb[:, 4:6], BB, axis=1), f32)
    fcwT = np.ascontiguousarray(np.asarray(inputs["fc_w"], f32).T)
    fcb = np.asarray(inputs["fc_b"], f32)
    fcb_nonzero = bool(np.any(fcb != 0))
    e1 = np.asarray(inputs["embed_w"], f32)[SEQBEGIN].astype(bfnp)
    l0 = np.empty((128, 2 * B), bfnp)
    for k in range(2):
        l0[:, k * B:(k + 1) * B] = np.tile(e1[k * 128:(k + 1) * 128][:, None], (1, B))
    allfacts = np.asarray(inputs["allfacts"], np.int32)
    questions = np.asarray(inputs["questions"], np.int32)

    in_maps = []
    for j in range(NCORE):
        m = dict(packs)
        m["embed"] = emb
        m["fcwT"] = np.ascontiguousarray(fcwT[:, j * VS:(j + 1) * VS]).astype(bfnp)
        if fcb_nonzero:
            m["fcb"] = np.ascontiguousarray(np.tile(fcb[None, j * VS:(j + 1) * VS], (B, 1)))
        m["l0"] = l0
        m["voff"] = np.full((B, 1), j * VS, f32)
        m["facts_idx"] = np.ascontiguousarray(
            allfacts[j * BB:(j + 1) * BB].reshape(NSEQ, FL).T.reshape(128, NTOK // 128))
        m["q_idx"] = np.ascontiguousarray(
            questions[j * BB:(j + 1) * BB].reshape(BB, QL).T.reshape(-1, 1))
        in_maps.append(m)
    return in_maps, fcb_nonzero


_CACHE = {}


def kernel(**inputs):
    alen = int(inputs["alen"])
    in_maps, fcb_nonzero = prep_inputs(inputs)
    key = (alen, fcb_nonzero)
    if key not in _CACHE:
        _CACHE[key] = build_nc(alen, fcb_nonzero)
    nc = _CACHE[key]
    res = run_bass_kernel_spmd(nc, in_maps, core_ids=list(range(NCORE)))
    out = np.concatenate([res.results[j]["out_logp"] for j in range(NCORE)], axis=1)
    return out.astype(np.float32)
